# revision 16
# baseline (speedup 1.0000x reference)
"""Self-contained Trainium2 Bass kernel for nn_AttnBlock (VAE-style attention).

Reference computation (per batch b):
  hn = GroupNorm32(x)                      # [C, N], stats per group of 16 chans
  q/k/v = W @ hn + b                       # 1x1 convs, C=512
  attn = softmax(q^T k / sqrt(C), axis=j)  # N=4096 spatial positions
  out  = x + Wp @ (v @ attn^T) + bp

Sharding: 8 cores = 2 batches x 4 query chunks of 1024. Each core receives
its batch's full image ROLLED so its local 1024 query columns come first,
making the SPMD program identical on every core (key order under softmax is
permutation invariant). GroupNorm + keys/values cover the full image on each
core; queries/residual only the local chunk.

Weight fusions (host-precomputed, exact):
  W2 = k_w^T (s q_w), b2 = k_w^T (s q_b):  scores^T = hn^T (W2 hn + b2)
       (the per-query offset bk.q is softmax-invariant and dropped).
  W3 = proj_w v_w:  out = (W3 hn E) * recip_den — V and proj collapse.
  bp_eff = proj_w v_b + proj_b.

fp8 strategy (validated numerically, ~4e-3 rel err vs the 2e-2 gate):
  x arrives bf16; groupnorm applied as hn8 = fp8e4m3(x*scl - shf).
  scores   = fp8 DoubleRow matmuls: hn8 (keys, stationary) x q28 (moving),
             contracting 2 slabs of 128 channels per matmul.
  q2 build = bf16 matmuls W2' @ x with W2' = W2*diag(scl) folded on device
             (q2 = W2 hn needs >fp8 weights; W2 in fp8 is numerically fatal),
             offset (W2 shf - b2) via 16 tiny matmuls on shf.
  vt3      = fp8 DoubleRow: hn8 x w3(fp8), requantized to fp8.
  E        = exp(scores - 1.0) on ScalarE, written as fp8 directly.
             (shift keeps exp <= ~103 < 240 = e4m3 max; softmax-invariant)
  AV       = fp8 DoubleRow: vt38 (stationary) x E8, accumulating in PSUM
             across all 32 key chunks.
  den      = fp8 DoubleRow ones-matmul on E8 -> PSUM broadcast colsums
             (replaces ~64 DVE/Pool adds with 16 cheap PE matmuls per tile).
  tail     = y = pos*recip(den) + bp_eff + x_local, f32 out.
"""

import numpy as np
import ml_dtypes

import concourse.bass as bass
import concourse.mybir as mybir
from concourse import bacc
import concourse.tile as tile
from concourse import bass_utils

P = 128          # partitions
C = 512          # channels
CS = C // P      # channel slabs (4)
G = 32           # groups
GS = C // G      # channels per group (16)
EPS = 1e-6
F32 = mybir.dt.float32
F8 = mybir.dt.float8e4
BF = mybir.dt.bfloat16
AL = mybir.AluOpType
AF = mybir.ActivationFunctionType
DR = mybir.MatmulPerfMode.DoubleRow

N_FULL = 4096    # spatial positions (64*64)
NQ = 1024        # local query chunk per core
IT = 512         # i-tile (queries per matmul free dim)
ESHIFT = -1.0    # exp bias: E = exp(scores - 1), softmax-invariant


def build_nc(n=N_FULL, nq=NQ, repeat=1, hw_loop=True,
             has_bp=False):
    """Build the per-core Bass program. All 8 cores run this same program."""
    njc = n // P          # key chunks of 128 (32)
    njp = njc // 2        # key chunk pairs (16)
    nit = nq // IT        # query tiles (2)

    nc = bacc.Bacc("TRN2", target_bir_lowering=False, debug=False)

    x_d = nc.dram_tensor("x", [C, n], BF, kind="ExternalInput")
    w2_d = nc.dram_tensor("w2_t", [C, C], BF, kind="ExternalInput")
    w3_d = nc.dram_tensor("w3_t", [C, C], F8, kind="ExternalInput")
    # small consts: per partition [bq(CS), bp(CS), gamma(CS), beta(CS), bo(P)]
    cpk_d = nc.dram_tensor("cpk", [P, 4 * CS + P + C], F32,
                           kind="ExternalInput")
    y_d = nc.dram_tensor("y", [C, nq], F32, kind="ExternalOutput")

    x_t = x_d.rearrange("(o p) n -> p o n", p=P)
    y_t = y_d.rearrange("(o p) n -> p o n", p=P)

    def rw(d):  # [C, C] -> [P, CS, C]  (contraction dim on partitions)
        return d.rearrange("(o p) c -> p o c", p=P)

    with tile.TileContext(nc) as tc:
        with (
            tc.tile_pool(name="consts", bufs=1) as consts,
            tc.tile_pool(name="xp", bufs=1) as xp,
            tc.tile_pool(name="persist", bufs=1) as persist,
            tc.tile_pool(name="ep", bufs=3) as ep,
            tc.tile_pool(name="yp", bufs=4) as yp,
            tc.tile_pool(name="psmm", bufs=3, space="PSUM") as psmm,
            tc.tile_pool(name="psacc", bufs=5, space="PSUM") as psacc,
        ):
            # ---- constants (outside the repeat loop) ----
            w2_sb = consts.tile([P, CS, C], BF, tag="w2")
            w38_sb = consts.tile([P, CS, C], F8, tag="w3")
            nc.scalar.dma_start(out=w2_sb, in_=rw(w2_d))
            nc.scalar.dma_start(out=w38_sb, in_=rw(w3_d))
            cpk_sb = consts.tile([P, 4 * CS + P + C], F32, tag="cpk")
            nc.scalar.dma_start(out=cpk_sb, in_=cpk_d[:, :])
            bq_sb = cpk_sb[:, 0 * CS:1 * CS]
            bp_sb = cpk_sb[:, 1 * CS:2 * CS]
            gam_sb = cpk_sb[:, 2 * CS:3 * CS]
            bet_sb = cpk_sb[:, 3 * CS:4 * CS]
            bo_sb = cpk_sb[:, 4 * CS:4 * CS + P]
            bpb_sb = cpk_sb[:, 4 * CS + P:4 * CS + P + C]
            eps_sb = consts.tile([P, 1], F32, tag="eps")
            nc.vector.memset(eps_sb, EPS)
            esh_sb = consts.tile([P, 1], F32, tag="esh")
            nc.vector.memset(esh_sb, ESHIFT)
            ones8 = consts.tile([P, 2, P], F8, tag="ones8")
            nc.vector.memset(ones8, 1.0)

            HN = n // 2   # x slab halves for DMA pipelining

            def body(pp):
                # ---- phase 1: load x (bf16), groupnorm per slab, write
                # normalized slab as fp8 (hn8) + fold scl into W2'. ----
                x_sb = xp.tile([P, CS, n], BF, tag=f"x{pp}", name="x_sb")
                hn8 = persist.tile([P, CS, n], F8, tag=f"hn8{pp}", name="hn8")
                w2p = persist.tile([P, CS, C], BF, tag=f"w2p{pp}", name="w2p")
                shfb = consts.tile([P, CS], BF, tag=f"shfb{pp}", name="shfb")
                engs = [nc.sync, nc.gpsimd]
                for po in range(CS):
                    for hh in range(2):
                        engs[hh].dma_start(
                            out=x_sb[:, po, hh * HN:(hh + 1) * HN],
                            in_=x_t[:, po, hh * HN:(hh + 1) * HN])
                    nchunk = n // 512
                    stats = consts.tile([P, nchunk, 6], F32, tag=f"st{po}_{pp}",
                                        name=f"st{po}")
                    xs4 = x_sb[:, po, :].rearrange(
                        "p (s f t) -> p s f t", f=256, t=2)
                    for s in range(nchunk):
                        nc.vector.bn_stats(out=stats[:, s, :],
                                           in_=xs4[:, s, :, 0])
                    mv = consts.tile([P, 2], F32, tag=f"mv{po}_{pp}", name=f"mv{po}")
                    nc.vector.bn_aggr(out=mv, in_=stats)
                    # var -> E[x^2] = mean*mean + var (in place)
                    nc.vector.scalar_tensor_tensor(
                        out=mv[:, 1:2], in0=mv[:, 0:1], scalar=mv[:, 0:1],
                        in1=mv[:, 1:2], op0=AL.mult, op1=AL.add)
                    # group-average within the slab: [P, 2] = BO^T @ mv
                    ps_st = psmm.tile([P, 2], F32, tag="ps_mm", name="ps_st")
                    nc.tensor.matmul(ps_st, bo_sb, mv, start=True, stop=True)
                    mvg = consts.tile([P, 2], F32, tag=f"mvg{po}_{pp}",
                                      name=f"mvg{po}")
                    nc.vector.tensor_copy(out=mvg, in_=ps_st)
                    gmean = mvg[:, 0:1]   # group E[x] per channel
                    gex2 = mvg[:, 1:2]    # group E[x^2] per channel
                    scl = consts.tile([P, 1], F32, tag=f"scl{po}_{pp}",
                                      name=f"scl{po}")
                    shf = consts.tile([P, 1], F32, tag=f"shf{po}_{pp}",
                                      name=f"shf{po}")
                    # scl <- -var = mean^2 - E[x^2]
                    nc.vector.scalar_tensor_tensor(
                        out=scl, in0=gmean, scalar=gmean, in1=gex2,
                        op0=AL.mult, op1=AL.subtract)
                    # sqrt(var + eps) via activation scale=-1
                    nc.scalar.activation(out=scl, in_=scl, func=AF.Sqrt,
                                         bias=eps_sb, scale=-1.0)
                    nc.vector.reciprocal(out=scl, in_=scl)
                    nc.vector.tensor_mul(out=scl, in0=scl,
                                         in1=gam_sb[:, po:po + 1])
                    # shf <- gmean*scl - beta = -(true shift)
                    nc.vector.scalar_tensor_tensor(
                        out=shf, in0=gmean, scalar=scl,
                        in1=bet_sb[:, po:po + 1], op0=AL.mult, op1=AL.subtract)
                    nc.vector.tensor_copy(out=shfb[:, po:po + 1], in_=shf)
                    # hn8 = fp8(x*scl - shf) on DVE (2 halves)
                    for hh in range(2):
                        nc.vector.tensor_scalar(
                            out=hn8[:, po, hh * HN:(hh + 1) * HN],
                            in0=x_sb[:, po, hh * HN:(hh + 1) * HN],
                            scalar1=scl, scalar2=shf,
                            op0=AL.mult, op1=AL.subtract)
                    # W2' slab = W2 slab * scl (bf16)
                    nc.vector.tensor_scalar_mul(
                        out=w2p[:, po, :], in0=w2_sb[:, po, :], scalar1=scl)

                # ---- phase 2a: q2 offset = W2^T shf - b2 via tiny matmuls
                off = consts.tile([P, CS], F32, tag=f"off{pp}", name="off")
                for cc in range(CS):
                    ps_o = psmm.tile([P, 1], F32, tag="ps_mm", name="ps_o")
                    for ks in range(CS):
                        nc.tensor.matmul(
                            ps_o, w2_sb[:, ks, cc * P:(cc + 1) * P],
                            shfb[:, ks:ks + 1],
                            start=(ks == 0), stop=(ks == CS - 1))
                    nc.vector.tensor_copy(out=off[:, cc:cc + 1], in_=ps_o)
                # negoff = bq - W2^T shf (Act Identity bias for q2)
                nc.vector.tensor_tensor(out=off, in0=bq_sb, in1=off,
                                        op=AL.subtract)

                # ---- phase 2b: q28 = fp8(W2' @ x - off) for local queries
                q28 = persist.tile([P, CS, nq], F8, tag=f"q28{pp}", name="q28")
                for cc in range(CS):
                    for it in range(nit):
                        isl = slice(it * IT, (it + 1) * IT)
                        ps = psmm.tile([P, IT], F32, tag="ps_mm", name="ps_q")
                        for ks in range(CS):
                            nc.tensor.matmul(
                                ps, w2p[:, ks, cc * P:(cc + 1) * P],
                                x_sb[:, ks, isl],
                                start=(ks == 0), stop=(ks == CS - 1))
                        nc.scalar.activation(
                            out=q28[:, cc, isl], in_=ps, func=AF.Identity,
                            bias=off[:, cc:cc + 1], scale=1.0)

                # ---- phase 2c: vt38[j, c] = fp8(hn^T W3^T), DoubleRow ----
                vt38 = persist.tile([P, njc, C], F8, tag=f"vt38{pp}", name="vt38")
                for jcg in range(njc):
                    ps = psmm.tile([P, C], F32, tag="ps_mm", name="ps_v")
                    for kp in range(2):
                        nc.tensor.matmul(
                            ps,
                            hn8[:, 2 * kp:2 * kp + 2, jcg * P:(jcg + 1) * P],
                            w38_sb[:, 2 * kp:2 * kp + 2, :],
                            start=(kp == 0), stop=(kp == 1), perf_mode=DR)
                    # fold bp_eff in: (sum_j E (vt3+bp))/den = out + bp
                    if has_bp:
                        nc.vector.tensor_tensor(
                            out=vt38[:, jcg, :], in0=ps, in1=bpb_sb, op=AL.add)
                    elif jcg % 2 == 0:
                        nc.vector.tensor_copy(out=vt38[:, jcg, :], in_=ps)
                    else:
                        nc.scalar.copy(out=vt38[:, jcg, :], in_=ps)

                # ---- phase 3: per query tile, one pass over all key chunk
                # pairs; scores/exp per 128-chunk, AV + den per pair, all
                # accumulating in PSUM ----
                for it in range(nit):
                    isl = slice(it * IT, (it + 1) * IT)
                    den_ps = psacc.tile([P, IT], F32, tag="acc", name="den")
                    pos = [psacc.tile([P, IT], F32, tag="acc",
                                      name=f"po{cc}")
                           for cc in range(CS)]
                    for g in range(njp):
                        e2 = ep.tile([P, 2, IT], F8, tag="e2", name="e2")
                        for hf in range(2):
                            jcg = 2 * g + hf
                            ps_s = psmm.tile([P, IT], F32, tag="ps_mm",
                                             name="ps_s")
                            for kp in range(2):
                                nc.tensor.matmul(
                                    ps_s,
                                    hn8[:, 2 * kp:2 * kp + 2,
                                        jcg * P:(jcg + 1) * P],
                                    q28[:, 2 * kp:2 * kp + 2, isl],
                                    start=(kp == 0), stop=(kp == 1),
                                    perf_mode=DR)
                            nc.scalar.activation(
                                out=e2[:, hf, :], in_=ps_s, func=AF.Exp,
                                bias=esh_sb, scale=1.0)
                        # den partial-sums broadcast: ones8^T @ e2
                        nc.tensor.matmul(
                            den_ps, ones8, e2,
                            start=(g == 0), stop=(g == njp - 1), perf_mode=DR)
                        # AV accumulate across the whole key loop
                        for cc in range(CS):
                            nc.tensor.matmul(
                                pos[cc],
                                vt38[:, 2 * g:2 * g + 2,
                                     cc * P:(cc + 1) * P],
                                e2,
                                start=(g == 0), stop=(g == njp - 1),
                                perf_mode=DR)

                    # tail: y = pos*recip(den) + bp_eff + x_local
                    recip = consts.tile([P, IT], F32, tag=f"recip{it}_{pp}",
                                        name=f"recip{it}")
                    nc.vector.reciprocal(out=recip, in_=den_ps)
                    for cc in range(CS):
                        yt = yp.tile([P, IT], F32, tag="yt", name="yt")
                        nc.vector.tensor_tensor(
                            out=yt, in0=pos[cc], in1=recip, op=AL.mult)
                        nc.gpsimd.tensor_tensor(
                            out=yt, in0=yt, in1=x_sb[:, cc, isl], op=AL.add)
                        engs[cc % 2].dma_start(out=y_t[:, cc, isl], in_=yt)

            if repeat == 1:
                body(0)
            elif not hw_loop:   # flat unroll for the timeline profiler
                for u in range(repeat):
                    body(u % 2)
            else:
                assert repeat % 4 == 0
                with tc.For_i(0, repeat // 4, 1):
                    for u in range(4):
                        body(u % 2)

    nc.compile()
    return nc


_NC_CACHE = {}


def _get_nc(n=N_FULL, nq=NQ, repeat=1, hw_loop=True, has_bp=False):
    key = (n, nq, repeat, hw_loop, has_bp)
    if key not in _NC_CACHE:
        _NC_CACHE[key] = build_nc(n, nq, repeat, hw_loop, has_bp)
    return _NC_CACHE[key]


def make_in_maps(x, q_w, q_b, k_w, k_b, v_w, v_b, proj_w, proj_b,
                 norm_gamma, norm_beta, n_cores=8):
    """Build per-core input dicts from the full problem inputs."""
    B = x.shape[0]
    n = x.shape[2] * x.shape[3]
    xf = np.ascontiguousarray(
        x.reshape(B, C, n).astype(ml_dtypes.bfloat16))
    scale = np.float64(C) ** -0.5
    w2 = k_w.astype(np.float64).T @ (q_w.astype(np.float64) * scale)
    b2 = k_w.astype(np.float64).T @ (q_b.astype(np.float64) * scale)
    w2_t = np.ascontiguousarray(w2.T.astype(ml_dtypes.bfloat16))
    w3 = proj_w.astype(np.float64) @ v_w.astype(np.float64)
    w3_t = np.ascontiguousarray(w3.T.astype(ml_dtypes.float8_e4m3))
    bq = b2.astype(np.float32)
    bp_eff = (proj_w.astype(np.float64) @ v_b.astype(np.float64)
              + proj_b.astype(np.float64)).astype(np.float32)
    # block-diagonal group-averaging matrix: 16x16 blocks of 1/16
    bo = np.zeros((P, P), np.float32)
    for g in range(P // GS):
        bo[g * GS:(g + 1) * GS, g * GS:(g + 1) * GS] = 1.0 / GS
    def r2h(v):  # [C] -> [P, CS] with c = o*P + p
        return np.ascontiguousarray(v.reshape(CS, P).T.astype(np.float32))
    bp_bcast = np.broadcast_to(bp_eff[None, :], (P, C)).astype(np.float32)
    cpk = np.concatenate(
        [r2h(bq), r2h(bp_eff),
         r2h(norm_gamma.astype(np.float32)), r2h(norm_beta.astype(np.float32)),
         bo, bp_bcast], axis=1)
    chunks = n_cores // B
    nq = n // chunks
    in_maps = []
    for g in range(n_cores):
        b, qc = divmod(g, chunks)
        xg = np.roll(xf[b], -qc * nq, axis=1)
        in_maps.append(dict(
            x=np.ascontiguousarray(xg), w2_t=w2_t, w3_t=w3_t, cpk=cpk))
    return in_maps


def kernel(**inputs):
    x = np.asarray(inputs["x"], np.float32)
    B, _, H, W = x.shape
    n = H * W
    chunks = 8 // B
    nq = n // chunks
    in_maps = make_in_maps(
        x, np.asarray(inputs["q_w"]), np.asarray(inputs["q_b"]),
        np.asarray(inputs["k_w"]), np.asarray(inputs["k_b"]),
        np.asarray(inputs["v_w"]), np.asarray(inputs["v_b"]),
        np.asarray(inputs["proj_w"]), np.asarray(inputs["proj_b"]),
        np.asarray(inputs["norm_gamma"]), np.asarray(inputs["norm_beta"]))
    bp_eff = (np.asarray(inputs["proj_w"], np.float64)
              @ np.asarray(inputs["v_b"], np.float64)
              + np.asarray(inputs["proj_b"], np.float64))
    nc = _get_nc(n, nq, has_bp=bool(np.abs(bp_eff).max() > 1e-7))
    res = bass_utils.run_bass_kernel_spmd(nc, in_maps, core_ids=list(range(8)))
    y = np.empty((B, C, n), np.float32)
    for g in range(8):
        b, qc = divmod(g, chunks)
        y[b][:, qc * nq:(qc + 1) * nq] = res.results[g]["y"]
    return y.reshape(B, C, H, W)


# revision 17
# speedup vs baseline: 1.0021x; 1.0021x over previous
"""Self-contained Trainium2 Bass kernel for nn_AttnBlock (VAE-style attention).

Reference computation (per batch b):
  hn = GroupNorm32(x)                      # [C, N], stats per group of 16 chans
  q/k/v = W @ hn + b                       # 1x1 convs, C=512
  attn = softmax(q^T k / sqrt(C), axis=j)  # N=4096 spatial positions
  out  = x + Wp @ (v @ attn^T) + bp

Sharding: 8 cores = 2 batches x 4 query chunks of 1024. Each core receives
its batch's full image ROLLED so its local 1024 query columns come first,
making the SPMD program identical on every core (key order under softmax is
permutation invariant). GroupNorm + keys/values cover the full image on each
core; queries/residual only the local chunk.

Weight fusions (host-precomputed, exact):
  W2 = k_w^T (s q_w), b2 = k_w^T (s q_b):  scores^T = hn^T (W2 hn + b2)
       (the per-query offset bk.q is softmax-invariant and dropped).
  W3 = proj_w v_w:  out = (W3 hn E) * recip_den — V and proj collapse.
  bp_eff = proj_w v_b + proj_b.

fp8 strategy (validated numerically, ~4e-3 rel err vs the 2e-2 gate):
  x arrives bf16; groupnorm applied as hn8 = fp8e4m3(x*scl - shf).
  scores   = fp8 DoubleRow matmuls: hn8 (keys, stationary) x q28 (moving),
             contracting 2 slabs of 128 channels per matmul.
  q2 build = bf16 matmuls W2' @ x with W2' = W2*diag(scl) folded on device
             (q2 = W2 hn needs >fp8 weights; W2 in fp8 is numerically fatal),
             offset (W2 shf - b2) via 16 tiny matmuls on shf.
  vt3      = fp8 DoubleRow: hn8 x w3(fp8), requantized to fp8.
  E        = exp(scores - 1.0) on ScalarE, written as fp8 directly.
             (shift keeps exp <= ~103 < 240 = e4m3 max; softmax-invariant)
  AV       = fp8 DoubleRow: vt38 (stationary) x E8, accumulating in PSUM
             across all 32 key chunks.
  den      = fp8 DoubleRow ones-matmul on E8 -> PSUM broadcast colsums
             (replaces ~64 DVE/Pool adds with 16 cheap PE matmuls per tile).
  tail     = y = pos*recip(den) + bp_eff + x_local, f32 out.
"""

import numpy as np
import ml_dtypes

import concourse.bass as bass
import concourse.mybir as mybir
from concourse import bacc
import concourse.tile as tile
from concourse import bass_utils

P = 128          # partitions
C = 512          # channels
CS = C // P      # channel slabs (4)
G = 32           # groups
GS = C // G      # channels per group (16)
EPS = 1e-6
F32 = mybir.dt.float32
F8 = mybir.dt.float8e4
BF = mybir.dt.bfloat16
AL = mybir.AluOpType
AF = mybir.ActivationFunctionType
DR = mybir.MatmulPerfMode.DoubleRow

N_FULL = 4096    # spatial positions (64*64)
NQ = 1024        # local query chunk per core
IT = 512         # i-tile (queries per matmul free dim)
ESHIFT = -1.0    # exp bias: E = exp(scores - 1), softmax-invariant


def build_nc(n=N_FULL, nq=NQ, repeat=1, hw_loop=True,
             has_bp=False):
    """Build the per-core Bass program. All 8 cores run this same program."""
    njc = n // P          # key chunks of 128 (32)
    njp = njc // 2        # key chunk pairs (16)
    nit = nq // IT        # query tiles (2)

    nc = bacc.Bacc("TRN2", target_bir_lowering=False, debug=False)

    x_d = nc.dram_tensor("x", [C, n], BF, kind="ExternalInput")
    w2_d = nc.dram_tensor("w2_t", [C, C], BF, kind="ExternalInput")
    w3_d = nc.dram_tensor("w3_t", [C, C], F8, kind="ExternalInput")
    # small consts: per partition [bq(CS), bp(CS), gamma(CS), beta(CS), bo(P)]
    cpk_d = nc.dram_tensor("cpk", [P, 4 * CS + P + C], F32,
                           kind="ExternalInput")
    y_d = nc.dram_tensor("y", [C, nq], F32, kind="ExternalOutput")

    x_t = x_d.rearrange("(o p) n -> p o n", p=P)
    y_t = y_d.rearrange("(o p) n -> p o n", p=P)

    def rw(d):  # [C, C] -> [P, CS, C]  (contraction dim on partitions)
        return d.rearrange("(o p) c -> p o c", p=P)

    with tile.TileContext(nc) as tc:
        with (
            tc.tile_pool(name="consts", bufs=1) as consts,
            tc.tile_pool(name="xp", bufs=1) as xp,
            tc.tile_pool(name="persist", bufs=1) as persist,
            tc.tile_pool(name="ep", bufs=3) as ep,
            tc.tile_pool(name="yp", bufs=4) as yp,
            tc.tile_pool(name="psmm", bufs=3, space="PSUM") as psmm,
            tc.tile_pool(name="psacc", bufs=5, space="PSUM") as psacc,
        ):
            # ---- constants (outside the repeat loop) ----
            w2_sb = consts.tile([P, CS, C], BF, tag="w2")
            w38_sb = consts.tile([P, CS, C], F8, tag="w3")
            nc.scalar.dma_start(out=w2_sb, in_=rw(w2_d))
            nc.scalar.dma_start(out=w38_sb, in_=rw(w3_d))
            cpk_sb = consts.tile([P, 4 * CS + P + C], F32, tag="cpk")
            nc.scalar.dma_start(out=cpk_sb, in_=cpk_d[:, :])
            bq_sb = cpk_sb[:, 0 * CS:1 * CS]
            bp_sb = cpk_sb[:, 1 * CS:2 * CS]
            gam_sb = cpk_sb[:, 2 * CS:3 * CS]
            bet_sb = cpk_sb[:, 3 * CS:4 * CS]
            bo_sb = cpk_sb[:, 4 * CS:4 * CS + P]
            bpb_sb = cpk_sb[:, 4 * CS + P:4 * CS + P + C]
            eps_sb = consts.tile([P, 1], F32, tag="eps")
            nc.vector.memset(eps_sb, EPS)
            esh_sb = consts.tile([P, 1], F32, tag="esh")
            nc.vector.memset(esh_sb, ESHIFT)
            ones8 = consts.tile([P, 2, P], F8, tag="ones8")
            nc.vector.memset(ones8, 1.0)

            HN = n // 2   # x slab halves for DMA pipelining

            def body(pp):
                # ---- phase 1: load x (bf16), groupnorm per slab, write
                # normalized slab as fp8 (hn8) + fold scl into W2'. ----
                x_sb = xp.tile([P, CS, n], BF, tag=f"x{pp}", name="x_sb")
                hn8 = persist.tile([P, CS, n], F8, tag=f"hn8{pp}", name="hn8")
                w2p = persist.tile([P, CS, C], BF, tag=f"w2p{pp}", name="w2p")
                shfb = consts.tile([P, CS], BF, tag=f"shfb{pp}", name="shfb")
                engs = [nc.sync, nc.gpsimd]
                for po in range(CS):
                    for hh in range(2):
                        engs[hh].dma_start(
                            out=x_sb[:, po, hh * HN:(hh + 1) * HN],
                            in_=x_t[:, po, hh * HN:(hh + 1) * HN])
                    nchunk = n // 512
                    stats = consts.tile([P, nchunk, 6], F32, tag=f"st{po}_{pp}",
                                        name=f"st{po}")
                    xs3 = x_sb[:, po, :].rearrange("p (s f) -> p s f", f=512)
                    for s in range(nchunk):
                        nc.vector.bn_stats(out=stats[:, s, :], in_=xs3[:, s, :])
                    mv = consts.tile([P, 2], F32, tag=f"mv{po}_{pp}", name=f"mv{po}")
                    nc.vector.bn_aggr(out=mv, in_=stats)
                    # var -> E[x^2] = mean*mean + var (in place)
                    nc.vector.scalar_tensor_tensor(
                        out=mv[:, 1:2], in0=mv[:, 0:1], scalar=mv[:, 0:1],
                        in1=mv[:, 1:2], op0=AL.mult, op1=AL.add)
                    # group-average within the slab: [P, 2] = BO^T @ mv
                    ps_st = psmm.tile([P, 2], F32, tag="ps_mm", name="ps_st")
                    nc.tensor.matmul(ps_st, bo_sb, mv, start=True, stop=True)
                    mvg = consts.tile([P, 2], F32, tag=f"mvg{po}_{pp}",
                                      name=f"mvg{po}")
                    nc.vector.tensor_copy(out=mvg, in_=ps_st)
                    gmean = mvg[:, 0:1]   # group E[x] per channel
                    gex2 = mvg[:, 1:2]    # group E[x^2] per channel
                    scl = consts.tile([P, 1], F32, tag=f"scl{po}_{pp}",
                                      name=f"scl{po}")
                    shf = consts.tile([P, 1], F32, tag=f"shf{po}_{pp}",
                                      name=f"shf{po}")
                    # scl <- -var = mean^2 - E[x^2]
                    nc.vector.scalar_tensor_tensor(
                        out=scl, in0=gmean, scalar=gmean, in1=gex2,
                        op0=AL.mult, op1=AL.subtract)
                    # sqrt(var + eps) via activation scale=-1
                    nc.scalar.activation(out=scl, in_=scl, func=AF.Sqrt,
                                         bias=eps_sb, scale=-1.0)
                    nc.vector.reciprocal(out=scl, in_=scl)
                    nc.vector.tensor_mul(out=scl, in0=scl,
                                         in1=gam_sb[:, po:po + 1])
                    # shf <- gmean*scl - beta = -(true shift)
                    nc.vector.scalar_tensor_tensor(
                        out=shf, in0=gmean, scalar=scl,
                        in1=bet_sb[:, po:po + 1], op0=AL.mult, op1=AL.subtract)
                    nc.vector.tensor_copy(out=shfb[:, po:po + 1], in_=shf)
                    # hn8 = fp8(x*scl - shf) on DVE (2 halves)
                    for hh in range(2):
                        nc.vector.tensor_scalar(
                            out=hn8[:, po, hh * HN:(hh + 1) * HN],
                            in0=x_sb[:, po, hh * HN:(hh + 1) * HN],
                            scalar1=scl, scalar2=shf,
                            op0=AL.mult, op1=AL.subtract)
                    # W2' slab = W2 slab * scl (bf16)
                    nc.vector.tensor_scalar_mul(
                        out=w2p[:, po, :], in0=w2_sb[:, po, :], scalar1=scl)

                # ---- phase 2a: q2 offset = W2^T shf - b2 via tiny matmuls
                off = consts.tile([P, CS], F32, tag=f"off{pp}", name="off")
                for cc in range(CS):
                    ps_o = psmm.tile([P, 1], F32, tag="ps_mm", name="ps_o")
                    for ks in range(CS):
                        nc.tensor.matmul(
                            ps_o, w2_sb[:, ks, cc * P:(cc + 1) * P],
                            shfb[:, ks:ks + 1],
                            start=(ks == 0), stop=(ks == CS - 1))
                    nc.vector.tensor_copy(out=off[:, cc:cc + 1], in_=ps_o)
                # negoff = bq - W2^T shf (Act Identity bias for q2)
                nc.vector.tensor_tensor(out=off, in0=bq_sb, in1=off,
                                        op=AL.subtract)

                # ---- phase 2b: q28 = fp8(W2' @ x - off) for local queries
                q28 = persist.tile([P, CS, nq], F8, tag=f"q28{pp}", name="q28")
                for cc in range(CS):
                    for it in range(nit):
                        isl = slice(it * IT, (it + 1) * IT)
                        ps = psmm.tile([P, IT], F32, tag="ps_mm", name="ps_q")
                        for ks in range(CS):
                            nc.tensor.matmul(
                                ps, w2p[:, ks, cc * P:(cc + 1) * P],
                                x_sb[:, ks, isl],
                                start=(ks == 0), stop=(ks == CS - 1))
                        nc.scalar.activation(
                            out=q28[:, cc, isl], in_=ps, func=AF.Identity,
                            bias=off[:, cc:cc + 1], scale=1.0)

                # ---- phase 2c: vt38[j, c] = fp8(hn^T W3^T), DoubleRow ----
                vt38 = persist.tile([P, njc, C], F8, tag=f"vt38{pp}", name="vt38")
                for jcg in range(njc):
                    ps = psmm.tile([P, C], F32, tag="ps_mm", name="ps_v")
                    for kp in range(2):
                        nc.tensor.matmul(
                            ps,
                            hn8[:, 2 * kp:2 * kp + 2, jcg * P:(jcg + 1) * P],
                            w38_sb[:, 2 * kp:2 * kp + 2, :],
                            start=(kp == 0), stop=(kp == 1), perf_mode=DR)
                    # fold bp_eff in: (sum_j E (vt3+bp))/den = out + bp
                    if has_bp:
                        nc.vector.tensor_tensor(
                            out=vt38[:, jcg, :], in0=ps, in1=bpb_sb, op=AL.add)
                    elif jcg % 2 == 0:
                        nc.vector.tensor_copy(out=vt38[:, jcg, :], in_=ps)
                    else:
                        nc.scalar.copy(out=vt38[:, jcg, :], in_=ps)

                # ---- phase 3: per query tile, one pass over all key chunk
                # pairs; scores/exp per 128-chunk, AV + den per pair, all
                # accumulating in PSUM ----
                for it in range(nit):
                    isl = slice(it * IT, (it + 1) * IT)
                    den_ps = psacc.tile([P, IT], F32, tag="acc", name="den")
                    pos = [psacc.tile([P, IT], F32, tag="acc",
                                      name=f"po{cc}")
                           for cc in range(CS)]
                    for g in range(njp):
                        e2 = ep.tile([P, 2, IT], F8, tag="e2", name="e2")
                        for hf in range(2):
                            jcg = 2 * g + hf
                            ps_s = psmm.tile([P, IT], F32, tag="ps_mm",
                                             name="ps_s")
                            for kp in range(2):
                                nc.tensor.matmul(
                                    ps_s,
                                    hn8[:, 2 * kp:2 * kp + 2,
                                        jcg * P:(jcg + 1) * P],
                                    q28[:, 2 * kp:2 * kp + 2, isl],
                                    start=(kp == 0), stop=(kp == 1),
                                    perf_mode=DR)
                            nc.scalar.activation(
                                out=e2[:, hf, :], in_=ps_s, func=AF.Exp,
                                bias=esh_sb, scale=1.0)
                        # den partial-sums broadcast: ones8^T @ e2
                        nc.tensor.matmul(
                            den_ps, ones8, e2,
                            start=(g == 0), stop=(g == njp - 1), perf_mode=DR)
                        # AV accumulate across the whole key loop
                        for cc in range(CS):
                            nc.tensor.matmul(
                                pos[cc],
                                vt38[:, 2 * g:2 * g + 2,
                                     cc * P:(cc + 1) * P],
                                e2,
                                start=(g == 0), stop=(g == njp - 1),
                                perf_mode=DR)

                    # tail: y = pos*recip(den) + bp_eff + x_local
                    recip = consts.tile([P, IT], F32, tag=f"recip{it}_{pp}",
                                        name=f"recip{it}")
                    nc.vector.reciprocal(out=recip, in_=den_ps)
                    for cc in range(CS):
                        yt = yp.tile([P, IT], F32, tag="yt", name="yt")
                        nc.vector.tensor_tensor(
                            out=yt, in0=pos[cc], in1=recip, op=AL.mult)
                        nc.gpsimd.tensor_tensor(
                            out=yt, in0=yt, in1=x_sb[:, cc, isl], op=AL.add)
                        engs[cc % 2].dma_start(out=y_t[:, cc, isl], in_=yt)

            if repeat == 1:
                body(0)
            elif not hw_loop:   # flat unroll for the timeline profiler
                for u in range(repeat):
                    body(u % 2)
            else:
                assert repeat % 4 == 0
                with tc.For_i(0, repeat // 4, 1):
                    for u in range(4):
                        body(u % 2)

    nc.compile()
    return nc


_NC_CACHE = {}


def _get_nc(n=N_FULL, nq=NQ, repeat=1, hw_loop=True, has_bp=False):
    key = (n, nq, repeat, hw_loop, has_bp)
    if key not in _NC_CACHE:
        _NC_CACHE[key] = build_nc(n, nq, repeat, hw_loop, has_bp)
    return _NC_CACHE[key]


def make_in_maps(x, q_w, q_b, k_w, k_b, v_w, v_b, proj_w, proj_b,
                 norm_gamma, norm_beta, n_cores=8):
    """Build per-core input dicts from the full problem inputs."""
    B = x.shape[0]
    n = x.shape[2] * x.shape[3]
    xf = np.ascontiguousarray(
        x.reshape(B, C, n).astype(ml_dtypes.bfloat16))
    scale = np.float64(C) ** -0.5
    w2 = k_w.astype(np.float64).T @ (q_w.astype(np.float64) * scale)
    b2 = k_w.astype(np.float64).T @ (q_b.astype(np.float64) * scale)
    w2_t = np.ascontiguousarray(w2.T.astype(ml_dtypes.bfloat16))
    w3 = proj_w.astype(np.float64) @ v_w.astype(np.float64)
    w3_t = np.ascontiguousarray(w3.T.astype(ml_dtypes.float8_e4m3))
    bq = b2.astype(np.float32)
    bp_eff = (proj_w.astype(np.float64) @ v_b.astype(np.float64)
              + proj_b.astype(np.float64)).astype(np.float32)
    # block-diagonal group-averaging matrix: 16x16 blocks of 1/16
    bo = np.zeros((P, P), np.float32)
    for g in range(P // GS):
        bo[g * GS:(g + 1) * GS, g * GS:(g + 1) * GS] = 1.0 / GS
    def r2h(v):  # [C] -> [P, CS] with c = o*P + p
        return np.ascontiguousarray(v.reshape(CS, P).T.astype(np.float32))
    bp_bcast = np.broadcast_to(bp_eff[None, :], (P, C)).astype(np.float32)
    cpk = np.concatenate(
        [r2h(bq), r2h(bp_eff),
         r2h(norm_gamma.astype(np.float32)), r2h(norm_beta.astype(np.float32)),
         bo, bp_bcast], axis=1)
    chunks = n_cores // B
    nq = n // chunks
    in_maps = []
    for g in range(n_cores):
        b, qc = divmod(g, chunks)
        xg = np.roll(xf[b], -qc * nq, axis=1)
        in_maps.append(dict(
            x=np.ascontiguousarray(xg), w2_t=w2_t, w3_t=w3_t, cpk=cpk))
    return in_maps


def kernel(**inputs):
    x = np.asarray(inputs["x"], np.float32)
    B, _, H, W = x.shape
    n = H * W
    chunks = 8 // B
    nq = n // chunks
    in_maps = make_in_maps(
        x, np.asarray(inputs["q_w"]), np.asarray(inputs["q_b"]),
        np.asarray(inputs["k_w"]), np.asarray(inputs["k_b"]),
        np.asarray(inputs["v_w"]), np.asarray(inputs["v_b"]),
        np.asarray(inputs["proj_w"]), np.asarray(inputs["proj_b"]),
        np.asarray(inputs["norm_gamma"]), np.asarray(inputs["norm_beta"]))
    bp_eff = (np.asarray(inputs["proj_w"], np.float64)
              @ np.asarray(inputs["v_b"], np.float64)
              + np.asarray(inputs["proj_b"], np.float64))
    nc = _get_nc(n, nq, has_bp=bool(np.abs(bp_eff).max() > 1e-7))
    res = bass_utils.run_bass_kernel_spmd(nc, in_maps, core_ids=list(range(8)))
    y = np.empty((B, C, n), np.float32)
    for g in range(8):
        b, qc = divmod(g, chunks)
        y[b][:, qc * nq:(qc + 1) * nq] = res.results[g]["y"]
    return y.reshape(B, C, H, W)


# revision 18
# speedup vs baseline: 1.0295x; 1.0273x over previous
"""Self-contained Trainium2 Bass kernel for nn_AttnBlock (VAE-style attention).

Reference computation (per batch b):
  hn = GroupNorm32(x)                      # [C, N], stats per group of 16 chans
  q/k/v = W @ hn + b                       # 1x1 convs, C=512
  attn = softmax(q^T k / sqrt(C), axis=j)  # N=4096 spatial positions
  out  = x + Wp @ (v @ attn^T) + bp

Sharding: 8 cores = 2 batches x 4 query chunks of 1024. Each core receives
its batch's full image ROLLED so its local 1024 query columns come first,
making the SPMD program identical on every core (key order under softmax is
permutation invariant). GroupNorm + keys/values cover the full image on each
core; queries/residual only the local chunk.

Weight fusions (host-precomputed, exact):
  W2 = k_w^T (s q_w), b2 = k_w^T (s q_b):  scores^T = hn^T (W2 hn + b2)
       (the per-query offset bk.q is softmax-invariant and dropped).
  W3 = proj_w v_w:  out = (W3 hn E) * recip_den — V and proj collapse.
  bp_eff = proj_w v_b + proj_b.

fp8 strategy (validated numerically, ~4e-3 rel err vs the 2e-2 gate):
  x arrives bf16; groupnorm applied as hn8 = fp8e4m3(x*scl - shf).
  scores   = fp8 DoubleRow matmuls: hn8 (keys, stationary) x q28 (moving),
             contracting 2 slabs of 128 channels per matmul.
  q2 build = bf16 matmuls W2' @ x with W2' = W2*diag(scl) folded on device
             (q2 = W2 hn needs >fp8 weights; W2 in fp8 is numerically fatal),
             offset (W2 shf - b2) via 16 tiny matmuls on shf.
  vt3      = fp8 DoubleRow: hn8 x w3(fp8), requantized to fp8.
  E        = exp(scores - 1.0) on ScalarE, written as fp8 directly.
             (shift keeps exp <= ~103 < 240 = e4m3 max; softmax-invariant)
  AV       = fp8 DoubleRow: vt38 (stationary) x E8, accumulating in PSUM
             across all 32 key chunks.
  den      = fp8 DoubleRow ones-matmul on E8 -> PSUM broadcast colsums
             (replaces ~64 DVE/Pool adds with 16 cheap PE matmuls per tile).
  tail     = y = pos*recip(den) + bp_eff + x_local, f32 out.
"""

import numpy as np
import ml_dtypes

import concourse.bass as bass
import concourse.mybir as mybir
from concourse import bacc
import concourse.tile as tile
from concourse import bass_utils

P = 128          # partitions
C = 512          # channels
CS = C // P      # channel slabs (4)
G = 32           # groups
GS = C // G      # channels per group (16)
EPS = 1e-6
F32 = mybir.dt.float32
F8 = mybir.dt.float8e4
BF = mybir.dt.bfloat16
AL = mybir.AluOpType
AF = mybir.ActivationFunctionType
DR = mybir.MatmulPerfMode.DoubleRow

N_FULL = 4096    # spatial positions (64*64)
NQ = 1024        # local query chunk per core
IT = 512         # i-tile (queries per matmul free dim)
ESHIFT = -1.0    # exp bias: E = exp(scores - 1), softmax-invariant


def build_nc(n=N_FULL, nq=NQ, repeat=1, hw_loop=True,
             has_bp=False):
    """Build the per-core Bass program. All 8 cores run this same program."""
    njc = n // P          # key chunks of 128 (32)
    njp = njc // 2        # key chunk pairs (16)
    nit = nq // IT        # query tiles (2)

    nc = bacc.Bacc("TRN2", target_bir_lowering=False, debug=False)

    x_d = nc.dram_tensor("x", [C, n], BF, kind="ExternalInput")
    w2_d = nc.dram_tensor("w2_t", [C, C], BF, kind="ExternalInput")
    w3_d = nc.dram_tensor("w3_t", [C, C], F8, kind="ExternalInput")
    # small consts: per partition [bq(CS), bp(CS), gamma(CS), beta(CS), bo(P)]
    cpk_d = nc.dram_tensor("cpk", [P, 4 * CS + P + C], F32,
                           kind="ExternalInput")
    y_d = nc.dram_tensor("y", [C, nq], F32, kind="ExternalOutput")

    x_t = x_d.rearrange("(o p) n -> p o n", p=P)
    y_t = y_d.rearrange("(o p) n -> p o n", p=P)

    def rw(d):  # [C, C] -> [P, CS, C]  (contraction dim on partitions)
        return d.rearrange("(o p) c -> p o c", p=P)

    with tile.TileContext(nc) as tc:
        with (
            tc.tile_pool(name="consts", bufs=1) as consts,
            tc.tile_pool(name="xp", bufs=1) as xp,
            tc.tile_pool(name="persist", bufs=1) as persist,
            tc.tile_pool(name="ep", bufs=3) as ep,
            tc.tile_pool(name="yp", bufs=4) as yp,
            tc.tile_pool(name="psmm", bufs=3, space="PSUM") as psmm,
            tc.tile_pool(name="psacc", bufs=5, space="PSUM") as psacc,
        ):
            # ---- constants (outside the repeat loop) ----
            w2_sb = consts.tile([P, CS, C], BF, tag="w2")
            w38_sb = consts.tile([P, CS, C], F8, tag="w3")
            nc.scalar.dma_start(out=w2_sb, in_=rw(w2_d))
            nc.scalar.dma_start(out=w38_sb, in_=rw(w3_d))
            cpk_sb = consts.tile([P, 4 * CS + P + C], F32, tag="cpk")
            nc.scalar.dma_start(out=cpk_sb, in_=cpk_d[:, :])
            bq_sb = cpk_sb[:, 0 * CS:1 * CS]
            bp_sb = cpk_sb[:, 1 * CS:2 * CS]
            gam_sb = cpk_sb[:, 2 * CS:3 * CS]
            bet_sb = cpk_sb[:, 3 * CS:4 * CS]
            bo_sb = cpk_sb[:, 4 * CS:4 * CS + P]
            bpb_sb = cpk_sb[:, 4 * CS + P:4 * CS + P + C]
            eps_sb = consts.tile([P, 1], F32, tag="eps")
            nc.vector.memset(eps_sb, EPS)
            esh_sb = consts.tile([P, 1], F32, tag="esh")
            nc.vector.memset(esh_sb, ESHIFT)
            ones8 = consts.tile([P, 2, P], F8, tag="ones8")
            nc.vector.memset(ones8, 1.0)

            HN = n // 2   # x slab halves for DMA pipelining

            def body(pp):
                # ---- phase 1: load x (bf16), groupnorm per slab, write
                # normalized slab as fp8 (hn8) + fold scl into W2'. ----
                x_sb = xp.tile([P, CS, n], BF, tag=f"x{pp}", name="x_sb")
                hn8 = persist.tile([P, CS, n], F8, tag=f"hn8{pp}", name="hn8")
                w2p = persist.tile([P, CS, C], BF, tag=f"w2p{pp}", name="w2p")
                shfb = consts.tile([P, CS], BF, tag=f"shfb{pp}", name="shfb")
                engs = [nc.sync, nc.gpsimd]
                for po in range(CS):
                    for hh in range(2):
                        engs[hh].dma_start(
                            out=x_sb[:, po, hh * HN:(hh + 1) * HN],
                            in_=x_t[:, po, hh * HN:(hh + 1) * HN])
                    nchunk = n // 512
                    stats = consts.tile([P, nchunk, 6], F32, tag=f"st{po}_{pp}",
                                        name=f"st{po}")
                    xs3 = x_sb[:, po, :].rearrange("p (s f) -> p s f", f=512)
                    for s in range(nchunk):
                        nc.vector.bn_stats(out=stats[:, s, :], in_=xs3[:, s, :])
                    mv = consts.tile([P, 2], F32, tag=f"mv{po}_{pp}", name=f"mv{po}")
                    nc.vector.bn_aggr(out=mv, in_=stats)
                    # var -> E[x^2] = mean*mean + var (in place)
                    nc.vector.scalar_tensor_tensor(
                        out=mv[:, 1:2], in0=mv[:, 0:1], scalar=mv[:, 0:1],
                        in1=mv[:, 1:2], op0=AL.mult, op1=AL.add)
                    # group-average within the slab: [P, 2] = BO^T @ mv
                    ps_st = psmm.tile([P, 2], F32, tag="ps_mm", name="ps_st")
                    nc.tensor.matmul(ps_st, bo_sb, mv, start=True, stop=True)
                    mvg = consts.tile([P, 2], F32, tag=f"mvg{po}_{pp}",
                                      name=f"mvg{po}")
                    nc.vector.tensor_copy(out=mvg, in_=ps_st)
                    gmean = mvg[:, 0:1]   # group E[x] per channel
                    gex2 = mvg[:, 1:2]    # group E[x^2] per channel
                    scl = consts.tile([P, 1], F32, tag=f"scl{po}_{pp}",
                                      name=f"scl{po}")
                    shf = consts.tile([P, 1], F32, tag=f"shf{po}_{pp}",
                                      name=f"shf{po}")
                    # scl <- -var = mean^2 - E[x^2]
                    nc.vector.scalar_tensor_tensor(
                        out=scl, in0=gmean, scalar=gmean, in1=gex2,
                        op0=AL.mult, op1=AL.subtract)
                    # sqrt(var + eps) via activation scale=-1
                    nc.scalar.activation(out=scl, in_=scl, func=AF.Sqrt,
                                         bias=eps_sb, scale=-1.0)
                    nc.vector.reciprocal(out=scl, in_=scl)
                    nc.vector.tensor_mul(out=scl, in0=scl,
                                         in1=gam_sb[:, po:po + 1])
                    # shf <- gmean*scl - beta = -(true shift)
                    nc.vector.scalar_tensor_tensor(
                        out=shf, in0=gmean, scalar=scl,
                        in1=bet_sb[:, po:po + 1], op0=AL.mult, op1=AL.subtract)
                    nc.vector.tensor_copy(out=shfb[:, po:po + 1], in_=shf)
                    # hn8 = fp8(x*scl - shf) on DVE (2 halves)
                    for hh in range(2):
                        nc.vector.tensor_scalar(
                            out=hn8[:, po, hh * HN:(hh + 1) * HN],
                            in0=x_sb[:, po, hh * HN:(hh + 1) * HN],
                            scalar1=scl, scalar2=shf,
                            op0=AL.mult, op1=AL.subtract)
                    # W2' slab = W2 slab * scl (bf16)
                    nc.vector.tensor_scalar_mul(
                        out=w2p[:, po, :], in0=w2_sb[:, po, :], scalar1=scl)

                # ---- phase 2a: q2 offset = W2^T shf - b2 via tiny matmuls
                off = consts.tile([P, CS], F32, tag=f"off{pp}", name="off")
                for cc in range(CS):
                    ps_o = psmm.tile([P, 1], F32, tag="ps_mm", name="ps_o")
                    for ks in range(CS):
                        nc.tensor.matmul(
                            ps_o, w2_sb[:, ks, cc * P:(cc + 1) * P],
                            shfb[:, ks:ks + 1],
                            start=(ks == 0), stop=(ks == CS - 1))
                    nc.vector.tensor_copy(out=off[:, cc:cc + 1], in_=ps_o)
                # negoff = bq - W2^T shf (Act Identity bias for q2)
                nc.vector.tensor_tensor(out=off, in0=bq_sb, in1=off,
                                        op=AL.subtract)

                # ---- phase 2b: q28 = fp8(W2' @ x - off) for local queries
                q28 = persist.tile([P, CS, nq], F8, tag=f"q28{pp}", name="q28")
                for cc in range(CS):
                    for it in range(nit):
                        isl = slice(it * IT, (it + 1) * IT)
                        ps = psmm.tile([P, IT], F32, tag="ps_mm", name="ps_q")
                        for ks in range(CS):
                            nc.tensor.matmul(
                                ps, w2p[:, ks, cc * P:(cc + 1) * P],
                                x_sb[:, ks, isl],
                                start=(ks == 0), stop=(ks == CS - 1))
                        nc.scalar.activation(
                            out=q28[:, cc, isl], in_=ps, func=AF.Identity,
                            bias=off[:, cc:cc + 1], scale=1.0)

                # ---- phase 2c: vt38[j, c] = fp8(hn^T W3^T), DoubleRow ----
                vt38 = persist.tile([P, njc, C], F8, tag=f"vt38{pp}", name="vt38")
                for jcg in range(njc):
                    ps = psmm.tile([P, C], F32, tag="ps_mm", name="ps_v")
                    for kp in range(2):
                        nc.tensor.matmul(
                            ps,
                            hn8[:, 2 * kp:2 * kp + 2, jcg * P:(jcg + 1) * P],
                            w38_sb[:, 2 * kp:2 * kp + 2, :],
                            start=(kp == 0), stop=(kp == 1), perf_mode=DR)
                    # fold bp_eff in: (sum_j E (vt3+bp))/den = out + bp
                    if has_bp:
                        nc.vector.tensor_tensor(
                            out=vt38[:, jcg, :], in0=ps, in1=bpb_sb, op=AL.add)
                    else:
                        nc.vector.tensor_copy(out=vt38[:, jcg, :], in_=ps)

                # ---- phase 3: per query tile, one pass over all key chunk
                # pairs; scores/exp per 128-chunk, AV + den per pair, all
                # accumulating in PSUM ----
                for it in range(nit):
                    isl = slice(it * IT, (it + 1) * IT)
                    den_ps = psacc.tile([P, IT], F32, tag="acc", name="den")
                    pos = [psacc.tile([P, IT], F32, tag="acc",
                                      name=f"po{cc}")
                           for cc in range(CS)]
                    for g in range(njp):
                        e2 = ep.tile([P, 2, IT], F8, tag="e2", name="e2")
                        for hf in range(2):
                            jcg = 2 * g + hf
                            ps_s = psmm.tile([P, IT], F32, tag="ps_mm",
                                             name="ps_s")
                            for kp in range(2):
                                nc.tensor.matmul(
                                    ps_s,
                                    hn8[:, 2 * kp:2 * kp + 2,
                                        jcg * P:(jcg + 1) * P],
                                    q28[:, 2 * kp:2 * kp + 2, isl],
                                    start=(kp == 0), stop=(kp == 1),
                                    perf_mode=DR)
                            nc.scalar.activation(
                                out=e2[:, hf, :], in_=ps_s, func=AF.Exp,
                                bias=esh_sb, scale=1.0)
                        # den partial-sums broadcast: ones8^T @ e2
                        nc.tensor.matmul(
                            den_ps, ones8, e2,
                            start=(g == 0), stop=(g == njp - 1), perf_mode=DR)
                        # AV accumulate across the whole key loop
                        for cc in range(CS):
                            nc.tensor.matmul(
                                pos[cc],
                                vt38[:, 2 * g:2 * g + 2,
                                     cc * P:(cc + 1) * P],
                                e2,
                                start=(g == 0), stop=(g == njp - 1),
                                perf_mode=DR)

                    # tail: y = pos*recip(den) + bp_eff + x_local
                    recip = consts.tile([P, IT], F32, tag=f"recip{it}_{pp}",
                                        name=f"recip{it}")
                    nc.vector.reciprocal(out=recip, in_=den_ps)
                    for cc in range(CS):
                        yt = yp.tile([P, IT], F32, tag="yt", name="yt")
                        nc.vector.tensor_tensor(
                            out=yt, in0=pos[cc], in1=recip, op=AL.mult)
                        nc.gpsimd.tensor_tensor(
                            out=yt, in0=yt, in1=x_sb[:, cc, isl], op=AL.add)
                        engs[cc % 2].dma_start(out=y_t[:, cc, isl], in_=yt)

            if repeat == 1:
                body(0)
            elif not hw_loop:   # flat unroll for the timeline profiler
                for u in range(repeat):
                    body(u % 2)
            else:
                assert repeat % 4 == 0
                with tc.For_i(0, repeat // 4, 1):
                    for u in range(4):
                        body(u % 2)

    nc.compile()
    return nc


_NC_CACHE = {}


def _get_nc(n=N_FULL, nq=NQ, repeat=1, hw_loop=True, has_bp=False):
    key = (n, nq, repeat, hw_loop, has_bp)
    if key not in _NC_CACHE:
        _NC_CACHE[key] = build_nc(n, nq, repeat, hw_loop, has_bp)
    return _NC_CACHE[key]


def make_in_maps(x, q_w, q_b, k_w, k_b, v_w, v_b, proj_w, proj_b,
                 norm_gamma, norm_beta, n_cores=8):
    """Build per-core input dicts from the full problem inputs."""
    B = x.shape[0]
    n = x.shape[2] * x.shape[3]
    xf = np.ascontiguousarray(
        x.reshape(B, C, n).astype(ml_dtypes.bfloat16))
    scale = np.float64(C) ** -0.5
    w2 = k_w.astype(np.float64).T @ (q_w.astype(np.float64) * scale)
    b2 = k_w.astype(np.float64).T @ (q_b.astype(np.float64) * scale)
    w2_t = np.ascontiguousarray(w2.T.astype(ml_dtypes.bfloat16))
    w3 = proj_w.astype(np.float64) @ v_w.astype(np.float64)
    w3_t = np.ascontiguousarray(w3.T.astype(ml_dtypes.float8_e4m3))
    bq = b2.astype(np.float32)
    bp_eff = (proj_w.astype(np.float64) @ v_b.astype(np.float64)
              + proj_b.astype(np.float64)).astype(np.float32)
    # block-diagonal group-averaging matrix: 16x16 blocks of 1/16
    bo = np.zeros((P, P), np.float32)
    for g in range(P // GS):
        bo[g * GS:(g + 1) * GS, g * GS:(g + 1) * GS] = 1.0 / GS
    def r2h(v):  # [C] -> [P, CS] with c = o*P + p
        return np.ascontiguousarray(v.reshape(CS, P).T.astype(np.float32))
    bp_bcast = np.broadcast_to(bp_eff[None, :], (P, C)).astype(np.float32)
    cpk = np.concatenate(
        [r2h(bq), r2h(bp_eff),
         r2h(norm_gamma.astype(np.float32)), r2h(norm_beta.astype(np.float32)),
         bo, bp_bcast], axis=1)
    chunks = n_cores // B
    nq = n // chunks
    in_maps = []
    for g in range(n_cores):
        b, qc = divmod(g, chunks)
        xg = np.roll(xf[b], -qc * nq, axis=1)
        in_maps.append(dict(
            x=np.ascontiguousarray(xg), w2_t=w2_t, w3_t=w3_t, cpk=cpk))
    return in_maps


def kernel(**inputs):
    x = np.asarray(inputs["x"], np.float32)
    B, _, H, W = x.shape
    n = H * W
    chunks = 8 // B
    nq = n // chunks
    in_maps = make_in_maps(
        x, np.asarray(inputs["q_w"]), np.asarray(inputs["q_b"]),
        np.asarray(inputs["k_w"]), np.asarray(inputs["k_b"]),
        np.asarray(inputs["v_w"]), np.asarray(inputs["v_b"]),
        np.asarray(inputs["proj_w"]), np.asarray(inputs["proj_b"]),
        np.asarray(inputs["norm_gamma"]), np.asarray(inputs["norm_beta"]))
    bp_eff = (np.asarray(inputs["proj_w"], np.float64)
              @ np.asarray(inputs["v_b"], np.float64)
              + np.asarray(inputs["proj_b"], np.float64))
    nc = _get_nc(n, nq, has_bp=bool(np.abs(bp_eff).max() > 1e-7))
    res = bass_utils.run_bass_kernel_spmd(nc, in_maps, core_ids=list(range(8)))
    y = np.empty((B, C, n), np.float32)
    for g in range(8):
        b, qc = divmod(g, chunks)
        y[b][:, qc * nq:(qc + 1) * nq] = res.results[g]["y"]
    return y.reshape(B, C, H, W)


# revision 19
# speedup vs baseline: 1.0297x; 1.0002x over previous
"""Self-contained Trainium2 Bass kernel for nn_AttnBlock (VAE-style attention).

Reference computation (per batch b):
  hn = GroupNorm32(x)                      # [C, N], stats per group of 16 chans
  q/k/v = W @ hn + b                       # 1x1 convs, C=512
  attn = softmax(q^T k / sqrt(C), axis=j)  # N=4096 spatial positions
  out  = x + Wp @ (v @ attn^T) + bp

Sharding: 8 cores = 2 batches x 4 query chunks of 1024. Each core receives
its batch's full image ROLLED so its local 1024 query columns come first,
making the SPMD program identical on every core (key order under softmax is
permutation invariant). GroupNorm + keys/values cover the full image on each
core; queries/residual only the local chunk.

Weight fusions (host-precomputed, exact):
  W2 = k_w^T (s q_w), b2 = k_w^T (s q_b):  scores^T = hn^T (W2 hn + b2)
       (the per-query offset bk.q is softmax-invariant and dropped).
  W3 = proj_w v_w:  out = (W3 hn E) * recip_den — V and proj collapse.
  bp_eff = proj_w v_b + proj_b.

fp8 strategy (validated numerically, ~4e-3 rel err vs the 2e-2 gate):
  x arrives bf16; groupnorm applied as hn8 = fp8e4m3(x*scl - shf).
  scores   = fp8 DoubleRow matmuls: hn8 (keys, stationary) x q28 (moving),
             contracting 2 slabs of 128 channels per matmul.
  q2 build = bf16 matmuls W2' @ x with W2' = W2*diag(scl) folded on device
             (q2 = W2 hn needs >fp8 weights; W2 in fp8 is numerically fatal),
             offset (W2 shf - b2) via 16 tiny matmuls on shf.
  vt3      = fp8 DoubleRow: hn8 x w3(fp8), requantized to fp8.
  E        = exp(scores - 1.0) on ScalarE, written as fp8 directly.
             (shift keeps exp <= ~103 < 240 = e4m3 max; softmax-invariant)
  AV       = fp8 DoubleRow: vt38 (stationary) x E8, accumulating in PSUM
             across all 32 key chunks.
  den      = fp8 DoubleRow ones-matmul on E8 -> PSUM broadcast colsums
             (replaces ~64 DVE/Pool adds with 16 cheap PE matmuls per tile).
  tail     = y = pos*recip(den) + bp_eff + x_local, f32 out.
"""

import numpy as np
import ml_dtypes

import concourse.bass as bass
import concourse.mybir as mybir
from concourse import bacc
import concourse.tile as tile
from concourse import bass_utils

P = 128          # partitions
C = 512          # channels
CS = C // P      # channel slabs (4)
G = 32           # groups
GS = C // G      # channels per group (16)
EPS = 1e-6
F32 = mybir.dt.float32
F8 = mybir.dt.float8e4
BF = mybir.dt.bfloat16
AL = mybir.AluOpType
AF = mybir.ActivationFunctionType
DR = mybir.MatmulPerfMode.DoubleRow

N_FULL = 4096    # spatial positions (64*64)
NQ = 1024        # local query chunk per core
IT = 512         # i-tile (queries per matmul free dim)
ESHIFT = -1.0    # exp bias: E = exp(scores - 1), softmax-invariant


def build_nc(n=N_FULL, nq=NQ, repeat=1, hw_loop=True,
             has_bp=False):
    """Build the per-core Bass program. All 8 cores run this same program."""
    njc = n // P          # key chunks of 128 (32)
    njp = njc // 2        # key chunk pairs (16)
    nit = nq // IT        # query tiles (2)

    nc = bacc.Bacc("TRN2", target_bir_lowering=False, debug=False)

    x_d = nc.dram_tensor("x", [C, n], BF, kind="ExternalInput")
    w2_d = nc.dram_tensor("w2_t", [C, C], BF, kind="ExternalInput")
    w3_d = nc.dram_tensor("w3_t", [C, C], F8, kind="ExternalInput")
    # small consts: per partition [bq(CS), bp(CS), gamma(CS), beta(CS), bo(P)]
    cpk_d = nc.dram_tensor("cpk", [P, 4 * CS + P + C], F32,
                           kind="ExternalInput")
    y_d = nc.dram_tensor("y", [C, nq], F32, kind="ExternalOutput")

    x_t = x_d.rearrange("(o p) n -> p o n", p=P)
    y_t = y_d.rearrange("(o p) n -> p o n", p=P)

    def rw(d):  # [C, C] -> [P, CS, C]  (contraction dim on partitions)
        return d.rearrange("(o p) c -> p o c", p=P)

    with tile.TileContext(nc) as tc:
        with (
            tc.tile_pool(name="consts", bufs=1) as consts,
            tc.tile_pool(name="xp", bufs=1) as xp,
            tc.tile_pool(name="persist", bufs=1) as persist,
            tc.tile_pool(name="ep", bufs=3) as ep,
            tc.tile_pool(name="yp", bufs=4) as yp,
            tc.tile_pool(name="psmm", bufs=3, space="PSUM") as psmm,
            tc.tile_pool(name="psacc", bufs=5, space="PSUM") as psacc,
        ):
            # ---- constants (outside the repeat loop) ----
            w2_sb = consts.tile([P, CS, C], BF, tag="w2")
            w38_sb = consts.tile([P, CS, C], F8, tag="w3")
            nc.scalar.dma_start(out=w2_sb, in_=rw(w2_d))
            nc.scalar.dma_start(out=w38_sb, in_=rw(w3_d))
            cpk_sb = consts.tile([P, 4 * CS + P + C], F32, tag="cpk")
            nc.scalar.dma_start(out=cpk_sb, in_=cpk_d[:, :])
            bq_sb = cpk_sb[:, 0 * CS:1 * CS]
            bp_sb = cpk_sb[:, 1 * CS:2 * CS]
            gam_sb = cpk_sb[:, 2 * CS:3 * CS]
            bet_sb = cpk_sb[:, 3 * CS:4 * CS]
            bo_sb = cpk_sb[:, 4 * CS:4 * CS + P]
            bpb_sb = cpk_sb[:, 4 * CS + P:4 * CS + P + C]
            eps_sb = consts.tile([P, 1], F32, tag="eps")
            nc.vector.memset(eps_sb, EPS)
            esh_sb = consts.tile([P, 1], F32, tag="esh")
            nc.vector.memset(esh_sb, ESHIFT)
            ones8 = consts.tile([P, 2, P], F8, tag="ones8")
            nc.vector.memset(ones8, 1.0)

            HN = n // 2   # x slab halves for DMA pipelining

            def body(pp):
                # ---- phase 1: load x (bf16), groupnorm per slab, write
                # normalized slab as fp8 (hn8) + fold scl into W2'. ----
                x_sb = xp.tile([P, CS, n], BF, tag=f"x{pp}", name="x_sb")
                hn8 = persist.tile([P, CS, n], F8, tag=f"hn8{pp}", name="hn8")
                w2p = persist.tile([P, CS, C], BF, tag=f"w2p{pp}", name="w2p")
                shfb = consts.tile([P, CS], BF, tag=f"shfb{pp}", name="shfb")
                engs = [nc.sync, nc.gpsimd]
                for po in range(CS):
                    for hh in range(2):
                        engs[hh].dma_start(
                            out=x_sb[:, po, hh * HN:(hh + 1) * HN],
                            in_=x_t[:, po, hh * HN:(hh + 1) * HN])
                    nchunk = n // 512
                    stats = consts.tile([P, nchunk, 6], F32, tag=f"st{po}_{pp}",
                                        name=f"st{po}")
                    xs3 = x_sb[:, po, :].rearrange("p (s f) -> p s f", f=512)
                    for s in range(nchunk):
                        nc.vector.bn_stats(out=stats[:, s, :], in_=xs3[:, s, :])
                    mv = consts.tile([P, 2], F32, tag=f"mv{po}_{pp}", name=f"mv{po}")
                    nc.vector.bn_aggr(out=mv, in_=stats)
                    # var -> E[x^2] = mean*mean + var (in place)
                    nc.vector.scalar_tensor_tensor(
                        out=mv[:, 1:2], in0=mv[:, 0:1], scalar=mv[:, 0:1],
                        in1=mv[:, 1:2], op0=AL.mult, op1=AL.add)
                    # group-average within the slab: [P, 2] = BO^T @ mv
                    ps_st = psmm.tile([P, 2], F32, tag="ps_mm", name="ps_st")
                    nc.tensor.matmul(ps_st, bo_sb, mv, start=True, stop=True)
                    mvg = consts.tile([P, 2], F32, tag=f"mvg{po}_{pp}",
                                      name=f"mvg{po}")
                    nc.vector.tensor_copy(out=mvg, in_=ps_st)
                    gmean = mvg[:, 0:1]   # group E[x] per channel
                    gex2 = mvg[:, 1:2]    # group E[x^2] per channel
                    scl = consts.tile([P, 1], F32, tag=f"scl{po}_{pp}",
                                      name=f"scl{po}")
                    shf = consts.tile([P, 1], F32, tag=f"shf{po}_{pp}",
                                      name=f"shf{po}")
                    # scl <- -var = mean^2 - E[x^2]
                    nc.vector.scalar_tensor_tensor(
                        out=scl, in0=gmean, scalar=gmean, in1=gex2,
                        op0=AL.mult, op1=AL.subtract)
                    # sqrt(var + eps) via activation scale=-1
                    nc.scalar.activation(out=scl, in_=scl, func=AF.Sqrt,
                                         bias=eps_sb, scale=-1.0)
                    nc.vector.reciprocal(out=scl, in_=scl)
                    nc.vector.tensor_mul(out=scl, in0=scl,
                                         in1=gam_sb[:, po:po + 1])
                    # shf <- gmean*scl - beta = -(true shift)
                    nc.vector.scalar_tensor_tensor(
                        out=shf, in0=gmean, scalar=scl,
                        in1=bet_sb[:, po:po + 1], op0=AL.mult, op1=AL.subtract)
                    nc.vector.tensor_copy(out=shfb[:, po:po + 1], in_=shf)
                    # hn8 = fp8(x*scl - shf): half on DVE, half on Act
                    # (Act is idle during the groupnorm phase)
                    nshf = consts.tile([P, 1], F32, tag=f"nshf{po}_{pp}",
                                       name=f"nshf{po}_{pp}")
                    nc.vector.tensor_scalar_mul(out=nshf, in0=shf,
                                                scalar1=-1.0)
                    nc.vector.tensor_scalar(
                        out=hn8[:, po, 0:HN],
                        in0=x_sb[:, po, 0:HN],
                        scalar1=scl, scalar2=shf,
                        op0=AL.mult, op1=AL.subtract)
                    nc.scalar.activation(
                        out=hn8[:, po, HN:n], in_=x_sb[:, po, HN:n],
                        func=AF.Identity, bias=nshf, scale=scl)
                    # W2' slab = W2 slab * scl (bf16)
                    nc.vector.tensor_scalar_mul(
                        out=w2p[:, po, :], in0=w2_sb[:, po, :], scalar1=scl)

                # ---- phase 2a: q2 offset = W2^T shf - b2 via tiny matmuls
                off = consts.tile([P, CS], F32, tag=f"off{pp}", name="off")
                for cc in range(CS):
                    ps_o = psmm.tile([P, 1], F32, tag="ps_mm", name="ps_o")
                    for ks in range(CS):
                        nc.tensor.matmul(
                            ps_o, w2_sb[:, ks, cc * P:(cc + 1) * P],
                            shfb[:, ks:ks + 1],
                            start=(ks == 0), stop=(ks == CS - 1))
                    nc.vector.tensor_copy(out=off[:, cc:cc + 1], in_=ps_o)
                # negoff = bq - W2^T shf (Act Identity bias for q2)
                nc.vector.tensor_tensor(out=off, in0=bq_sb, in1=off,
                                        op=AL.subtract)

                # ---- phase 2b: q28 = fp8(W2' @ x - off) for local queries
                q28 = persist.tile([P, CS, nq], F8, tag=f"q28{pp}", name="q28")
                for cc in range(CS):
                    for it in range(nit):
                        isl = slice(it * IT, (it + 1) * IT)
                        ps = psmm.tile([P, IT], F32, tag="ps_mm", name="ps_q")
                        for ks in range(CS):
                            nc.tensor.matmul(
                                ps, w2p[:, ks, cc * P:(cc + 1) * P],
                                x_sb[:, ks, isl],
                                start=(ks == 0), stop=(ks == CS - 1))
                        nc.scalar.activation(
                            out=q28[:, cc, isl], in_=ps, func=AF.Identity,
                            bias=off[:, cc:cc + 1], scale=1.0)

                # ---- phase 2c: vt38[j, c] = fp8(hn^T W3^T), DoubleRow ----
                vt38 = persist.tile([P, njc, C], F8, tag=f"vt38{pp}", name="vt38")
                for jcg in range(njc):
                    ps = psmm.tile([P, C], F32, tag="ps_mm", name="ps_v")
                    for kp in range(2):
                        nc.tensor.matmul(
                            ps,
                            hn8[:, 2 * kp:2 * kp + 2, jcg * P:(jcg + 1) * P],
                            w38_sb[:, 2 * kp:2 * kp + 2, :],
                            start=(kp == 0), stop=(kp == 1), perf_mode=DR)
                    # fold bp_eff in: (sum_j E (vt3+bp))/den = out + bp
                    if has_bp:
                        nc.vector.tensor_tensor(
                            out=vt38[:, jcg, :], in0=ps, in1=bpb_sb, op=AL.add)
                    else:
                        nc.vector.tensor_copy(out=vt38[:, jcg, :], in_=ps)

                # ---- phase 3: per query tile, one pass over all key chunk
                # pairs; scores/exp per 128-chunk, AV + den per pair, all
                # accumulating in PSUM ----
                for it in range(nit):
                    isl = slice(it * IT, (it + 1) * IT)
                    den_ps = psacc.tile([P, IT], F32, tag="acc", name="den")
                    pos = [psacc.tile([P, IT], F32, tag="acc",
                                      name=f"po{cc}")
                           for cc in range(CS)]
                    for g in range(njp):
                        e2 = ep.tile([P, 2, IT], F8, tag="e2", name="e2")
                        for hf in range(2):
                            jcg = 2 * g + hf
                            ps_s = psmm.tile([P, IT], F32, tag="ps_mm",
                                             name="ps_s")
                            for kp in range(2):
                                nc.tensor.matmul(
                                    ps_s,
                                    hn8[:, 2 * kp:2 * kp + 2,
                                        jcg * P:(jcg + 1) * P],
                                    q28[:, 2 * kp:2 * kp + 2, isl],
                                    start=(kp == 0), stop=(kp == 1),
                                    perf_mode=DR)
                            nc.scalar.activation(
                                out=e2[:, hf, :], in_=ps_s, func=AF.Exp,
                                bias=esh_sb, scale=1.0)
                        # den partial-sums broadcast: ones8^T @ e2
                        nc.tensor.matmul(
                            den_ps, ones8, e2,
                            start=(g == 0), stop=(g == njp - 1), perf_mode=DR)
                        # AV accumulate across the whole key loop
                        for cc in range(CS):
                            nc.tensor.matmul(
                                pos[cc],
                                vt38[:, 2 * g:2 * g + 2,
                                     cc * P:(cc + 1) * P],
                                e2,
                                start=(g == 0), stop=(g == njp - 1),
                                perf_mode=DR)

                    # tail: y = pos*recip(den) + bp_eff + x_local
                    recip = consts.tile([P, IT], F32, tag=f"recip{it}_{pp}",
                                        name=f"recip{it}")
                    nc.vector.reciprocal(out=recip, in_=den_ps)
                    for cc in range(CS):
                        yt = yp.tile([P, IT], F32, tag="yt", name="yt")
                        nc.vector.tensor_tensor(
                            out=yt, in0=pos[cc], in1=recip, op=AL.mult)
                        nc.gpsimd.tensor_tensor(
                            out=yt, in0=yt, in1=x_sb[:, cc, isl], op=AL.add)
                        engs[cc % 2].dma_start(out=y_t[:, cc, isl], in_=yt)

            if repeat == 1:
                body(0)
            elif not hw_loop:   # flat unroll for the timeline profiler
                for u in range(repeat):
                    body(u % 2)
            else:
                assert repeat % 4 == 0
                with tc.For_i(0, repeat // 4, 1):
                    for u in range(4):
                        body(u % 2)

    nc.compile()
    return nc


_NC_CACHE = {}


def _get_nc(n=N_FULL, nq=NQ, repeat=1, hw_loop=True, has_bp=False):
    key = (n, nq, repeat, hw_loop, has_bp)
    if key not in _NC_CACHE:
        _NC_CACHE[key] = build_nc(n, nq, repeat, hw_loop, has_bp)
    return _NC_CACHE[key]


def make_in_maps(x, q_w, q_b, k_w, k_b, v_w, v_b, proj_w, proj_b,
                 norm_gamma, norm_beta, n_cores=8):
    """Build per-core input dicts from the full problem inputs."""
    B = x.shape[0]
    n = x.shape[2] * x.shape[3]
    xf = np.ascontiguousarray(
        x.reshape(B, C, n).astype(ml_dtypes.bfloat16))
    scale = np.float64(C) ** -0.5
    w2 = k_w.astype(np.float64).T @ (q_w.astype(np.float64) * scale)
    b2 = k_w.astype(np.float64).T @ (q_b.astype(np.float64) * scale)
    w2_t = np.ascontiguousarray(w2.T.astype(ml_dtypes.bfloat16))
    w3 = proj_w.astype(np.float64) @ v_w.astype(np.float64)
    w3_t = np.ascontiguousarray(w3.T.astype(ml_dtypes.float8_e4m3))
    bq = b2.astype(np.float32)
    bp_eff = (proj_w.astype(np.float64) @ v_b.astype(np.float64)
              + proj_b.astype(np.float64)).astype(np.float32)
    # block-diagonal group-averaging matrix: 16x16 blocks of 1/16
    bo = np.zeros((P, P), np.float32)
    for g in range(P // GS):
        bo[g * GS:(g + 1) * GS, g * GS:(g + 1) * GS] = 1.0 / GS
    def r2h(v):  # [C] -> [P, CS] with c = o*P + p
        return np.ascontiguousarray(v.reshape(CS, P).T.astype(np.float32))
    bp_bcast = np.broadcast_to(bp_eff[None, :], (P, C)).astype(np.float32)
    cpk = np.concatenate(
        [r2h(bq), r2h(bp_eff),
         r2h(norm_gamma.astype(np.float32)), r2h(norm_beta.astype(np.float32)),
         bo, bp_bcast], axis=1)
    chunks = n_cores // B
    nq = n // chunks
    in_maps = []
    for g in range(n_cores):
        b, qc = divmod(g, chunks)
        xg = np.roll(xf[b], -qc * nq, axis=1)
        in_maps.append(dict(
            x=np.ascontiguousarray(xg), w2_t=w2_t, w3_t=w3_t, cpk=cpk))
    return in_maps


def kernel(**inputs):
    x = np.asarray(inputs["x"], np.float32)
    B, _, H, W = x.shape
    n = H * W
    chunks = 8 // B
    nq = n // chunks
    in_maps = make_in_maps(
        x, np.asarray(inputs["q_w"]), np.asarray(inputs["q_b"]),
        np.asarray(inputs["k_w"]), np.asarray(inputs["k_b"]),
        np.asarray(inputs["v_w"]), np.asarray(inputs["v_b"]),
        np.asarray(inputs["proj_w"]), np.asarray(inputs["proj_b"]),
        np.asarray(inputs["norm_gamma"]), np.asarray(inputs["norm_beta"]))
    bp_eff = (np.asarray(inputs["proj_w"], np.float64)
              @ np.asarray(inputs["v_b"], np.float64)
              + np.asarray(inputs["proj_b"], np.float64))
    nc = _get_nc(n, nq, has_bp=bool(np.abs(bp_eff).max() > 1e-7))
    res = bass_utils.run_bass_kernel_spmd(nc, in_maps, core_ids=list(range(8)))
    y = np.empty((B, C, n), np.float32)
    for g in range(8):
        b, qc = divmod(g, chunks)
        y[b][:, qc * nq:(qc + 1) * nq] = res.results[g]["y"]
    return y.reshape(B, C, H, W)
